# revision 12
# baseline (speedup 1.0000x reference)
"""BitLinear (BitNet b1.58 ternary-weight linear) Trainium2 kernel.

Reference computation:
    scale = mean(|w|)                      # global scalar over the FULL weight
    w_q   = round(clip(w / (scale+1e-8), -1, 1)) * scale    # ternary {-1,0,1}*scale
    out   = einsum('bsi,oi->bso', x, w_q)  # x @ w_q.T

Sharding (8 NeuronCores, tensor-parallel on out_features):
    core c receives:
      xt  [4096, 4096] bf16  = x.reshape(4096,4096).T   (replicated; [d_in, tok])
      wt  [4096,  512] f32   = w.T[:, c*512:(c+1)*512]  ([d_in, d_out/8] shard)
    and produces:
      out [4096,  512] f32   = (x @ w_q.T)[:, c*512:(c+1)*512]

    The global scale needs the sum of |w| over ALL shards, so each core
    reduces its own shard and a 512-byte AllGather exchanges the per-partition
    partials; a ones-matmul then reduces across ranks+partitions AND
    broadcasts the total to all 128 partitions.

Device pipeline per core:
  1. DMA wt shard into SBUF (resident, prioritized over x prefetch),
     per-128-row k-tile reduce sum(|w|) -> per-partition partials.
  2. AllGather partials across the 8 cores; reduce + ones-matmul broadcast.
  3. thresh = 0.5*(scale+eps); ternary-quantize the shard to bf16 in 2 DVE
     ops per k-tile, producing the NEGATED ternary pattern
     (w < -thresh) - (w > thresh); the negation is undone by multiplying
     the output by -scale (both steps are exact, so no precision is lost).
  4. 1024 accumulating matmuls: stationary = x.T tile [128k x 128t] (bf16),
     moving = quantized w.T k-slab [128k x 512o], accumulated over the 32
     k-tiles into PSUM banks (one per 128-token tile). Token tiles are
     processed in groups of 4 banks with the other 4 banks evacuating
     concurrently (ping-pong), so the PE never stalls on PSUM slots;
     evacuation is a DVE copy with a fused multiply by -scale.

Numerics: x is rounded to bf16 once (host side); everything else accumulates
in fp32 (PSUM) and the ternary weights are exact in bf16, so the end-to-end
error is ~1.7e-3 relative (bf16 input rounding), far inside the usual gates.
"""

import numpy as np
import ml_dtypes

import concourse.bass as bass
import concourse.bacc as bacc
import concourse.mybir as mybir
import concourse.tile as tile
from concourse.bass_utils import run_bass_kernel_spmd
from concourse.tile_rust import add_dep_helper

# Problem geometry (hardcoded per the contract).
B, S = 2, 2048
D_IN = 4096
D_OUT = 4096
N_CORES = 8

P = 128                      # SBUF/PSUM partitions
TOK = B * S                  # 4096 tokens
O_SHARD = D_OUT // N_CORES   # 512 output features per core
KT = D_IN // P               # 32 contraction k-tiles
TT = TOK // P                # 32 token tiles
NBANKS = 4                   # PSUM banks per token-tile group (4+4 ping-pong)
NG = TT // NBANKS            # 8 token-tile groups
GCOLS = P * NBANKS           # 512 tokens per group

F32 = mybir.dt.float32
BF16 = mybir.dt.bfloat16

EPS = np.float32(1e-8)
HALF_EPS = float(np.float32(0.5) * EPS)          # exact
NEG_INV_N = float(-np.float32(2.0 ** -24))       # -1/(4096*4096), exact
HALF_INV_N = float(np.float32(2.0 ** -25))


def _build_program():
    """Build and compile the per-core Bass program (identical on all cores)."""
    nc = bacc.Bacc("TRN2", target_bir_lowering=False, debug=False,
                   num_devices=N_CORES)

    xt = nc.dram_tensor("xt", [D_IN, TOK], BF16, kind="ExternalInput")
    wt = nc.dram_tensor("wt", [D_IN, O_SHARD], F32, kind="ExternalInput")
    out = nc.dram_tensor("out", [TOK, O_SHARD], F32, kind="ExternalOutput")

    rg = [list(range(N_CORES))]

    with tile.TileContext(nc) as tc:
        with (
            tc.tile_pool(name="const", bufs=1) as const,
            tc.tile_pool(name="wf", bufs=1) as wf,
            tc.tile_pool(name="wq", bufs=1) as wqp,
            tc.tile_pool(name="small", bufs=1) as small,
            tc.tile_pool(name="qtmp", bufs=4) as qtmp,
            tc.tile_pool(name="xp", bufs=8) as xp,
            tc.tile_pool(name="op", bufs=4) as op,
            tc.tile_pool(name="ps", bufs=8, space="PSUM") as ps,
            tc.tile_pool(name="dram", bufs=1, space="DRAM") as dram,
        ):
            ones_sb = const.tile([P, P], F32)
            nc.vector.memset(ones_sb[:], 1.0)

            # ---- phase 1: local sum(|w|) over the shard --------------------
            wt_sb = wf.tile([P, KT, O_SHARD], F32)       # resident fp32 shard
            partials = small.tile([P, KT], F32)
            w_dmas = []
            for k in range(KT):
                w_dmas.append(
                    nc.sync.dma_start(wt_sb[:, k, :], wt[k * P:(k + 1) * P, :]))
                nc.vector.tensor_reduce(
                    partials[:, k:k + 1], wt_sb[:, k, :],
                    axis=mybir.AxisListType.X, op=mybir.AluOpType.add,
                    apply_absolute_value=True,
                )
            partial1 = small.tile([P, 1], F32)
            nc.vector.tensor_reduce(
                partial1[:, 0:1], partials[:, :],
                axis=mybir.AxisListType.X, op=mybir.AluOpType.add,
            )

            # ---- AllGather the per-partition partials across the 8 cores ---
            # (AllGather's latency floor on 8 cores is ~half an AllReduce's;
            # the 8-way sum is folded into the ones-matmul below.)
            ag_in = dram.tile([P, 1], F32)
            ag_out = dram.tile([N_CORES * P, 1], F32)   # rank-major on dim 0
            nc.sync.dma_start(ag_in[:], partial1[:, 0:1])
            nc.gpsimd.collective_compute(
                "AllGather", mybir.AluOpType.bypass, replica_groups=rg,
                ins=[ag_in.opt()], outs=[ag_out.opt()],
            )
            # partition p <- the 8 ranks' values for partition p: [128, 8]
            gpart = small.tile([P, N_CORES], F32)
            nc.sync.dma_start(
                gpart[:, :], ag_out.opt().rearrange("(r p) c -> p (r c)", p=P))

            # reduce 8 ranks -> per-partition totals
            gpart1 = small.tile([P, 1], F32)
            nc.vector.tensor_reduce(
                gpart1[:, 0:1], gpart[:, :],
                axis=mybir.AxisListType.X, op=mybir.AluOpType.add)

            # reduce across partitions AND broadcast: ones[128,128].T @ gpart1
            psB = ps.tile([P, 512], F32, tag="acc", name="ps_bcast")
            nc.tensor.matmul(psB[:, 0:1], ones_sb[:, :], gpart1[:, 0:1],
                             start=True, stop=True)

            nscale_sb = small.tile([P, 1], F32)
            thresh_sb = small.tile([P, 1], F32)
            nthresh_sb = small.tile([P, 1], F32)
            # -scale = total * -2^-24 (exact); thresh = 0.5*(scale+eps)
            # computed as total*2^-25 + eps/2, bit-identical to the reference
            # (power-of-2 scaling commutes with fp32 rounding).
            nc.vector.tensor_scalar_mul(nscale_sb[:, 0:1], psB[:, 0:1], NEG_INV_N)
            nc.vector.tensor_scalar(
                thresh_sb[:, 0:1], psB[:, 0:1], HALF_INV_N, HALF_EPS,
                mybir.AluOpType.mult, mybir.AluOpType.add,
            )
            nc.vector.tensor_scalar_mul(nthresh_sb[:, 0:1], thresh_sb[:, 0:1], -1.0)

            # ---- ternary quantize shard -> bf16 NEGATED {-1, 0, +1} --------
            # wq = (w < -thresh) - (w > thresh) = -ternary(w); undone by -scale.
            wq_sb = wqp.tile([P, KT, O_SHARD], BF16)     # resident ternary shard
            for k in range(KT):
                pos = qtmp.tile([P, O_SHARD], BF16, tag="pos", name=f"pos_{k}")
                nc.vector.tensor_scalar(
                    pos[:], wt_sb[:, k, :], thresh_sb[:, 0:1], None,
                    mybir.AluOpType.is_gt,
                )
                nc.vector.scalar_tensor_tensor(
                    wq_sb[:, k, :], wt_sb[:, k, :], nthresh_sb[:, 0:1], pos[:],
                    mybir.AluOpType.is_lt, mybir.AluOpType.subtract,
                )

            # ---- main matmul: out[t, o] = sum_k xt[k, t] * wq[k, o] --------
            first_group = True
            for g in range(NG):
                psums = [ps.tile([P, 512], F32, tag="acc", name=f"acc_{g}_{t}")
                         for t in range(NBANKS)]
                for k in range(KT):
                    xt_t = xp.tile([P, GCOLS], BF16, tag="xt", name=f"xt_{g}_{k}")
                    xd = nc.sync.dma_start(
                        xt_t[:],
                        xt[k * P:(k + 1) * P, g * GCOLS:(g + 1) * GCOLS],
                    )
                    if first_group:
                        # keep the x prefetch out of the weight-shard DMA's
                        # way: the scale AllGather (and everything after it)
                        # is serialized behind the w DMAs, while x has ~60us
                        # of slack before the matmuls start.
                        add_dep_helper(xd.ins, w_dmas[-1].ins, True,
                                       "prioritize w shard DMA")
                    for t in range(NBANKS):
                        nc.tensor.matmul(
                            psums[t][:, :O_SHARD],
                            xt_t[:, t * P:(t + 1) * P],
                            wq_sb[:, k, :],
                            start=(k == 0), stop=(k == KT - 1),
                        )
                first_group = False
                for t in range(NBANKS):
                    ot = op.tile([P, O_SHARD], F32, tag="ot", name=f"ot_{g}_{t}")
                    nc.vector.tensor_scalar_mul(
                        ot[:], psums[t][:, :O_SHARD], nscale_sb[:, 0:1])
                    row = (g * NBANKS + t) * P
                    nc.sync.dma_start(out[row:row + P, :], ot[:])

    nc.compile()
    return nc


_NC_CACHE = None


def _get_program():
    global _NC_CACHE
    if _NC_CACHE is None:
        _NC_CACHE = _build_program()
    return _NC_CACHE


def _make_in_maps(input: np.ndarray, weight: np.ndarray):
    input = np.asarray(input, dtype=np.float32)
    weight = np.asarray(weight, dtype=np.float32)
    x2d = np.ascontiguousarray(input.reshape(TOK, D_IN))
    xt_np = np.ascontiguousarray(x2d.T).astype(ml_dtypes.bfloat16)
    wT = np.ascontiguousarray(weight.T)          # [d_in, d_out] fp32
    in_maps = []
    for c in range(N_CORES):
        in_maps.append({
            "xt": xt_np,
            "wt": np.ascontiguousarray(wT[:, c * O_SHARD:(c + 1) * O_SHARD]),
        })
    return in_maps


def run_device(input: np.ndarray, weight: np.ndarray, **spmd_kwargs):
    """Run the sharded kernel; returns (full_output, BassKernelResults)."""
    nc = _get_program()
    in_maps = _make_in_maps(input, weight)
    res = run_bass_kernel_spmd(nc, in_maps, list(range(N_CORES)), **spmd_kwargs)
    shards = [res.results[c]["out"] for c in range(N_CORES)]
    full = np.concatenate(shards, axis=1).reshape(B, S, D_OUT)
    return np.ascontiguousarray(full.astype(np.float32)), res


def kernel(input: np.ndarray, weight: np.ndarray) -> np.ndarray:
    out, _ = run_device(input, weight)
    return out


# revision 14
# speedup vs baseline: 1.2135x; 1.2135x over previous
"""BitLinear (BitNet b1.58 ternary-weight linear) Trainium2 kernel.

Reference computation:
    scale = mean(|w|)                      # global scalar over the FULL weight
    w_q   = round(clip(w / (scale+1e-8), -1, 1)) * scale    # ternary {-1,0,1}*scale
    out   = einsum('bsi,oi->bso', x, w_q)  # x @ w_q.T

Sharding (8 NeuronCores, tensor-parallel on out_features):
    core c receives:
      xt  [4096, 4096] bf16  = x.reshape(4096,4096).T   (replicated; [d_in, tok])
      wt  [4096,  512] f32   = w.T[:, c*512:(c+1)*512]  ([d_in, d_out/8] shard)
    and produces:
      out [4096,  512] f32   = (x @ w_q.T)[:, c*512:(c+1)*512]

Two collective-free launches instead of one collective kernel:
    A NEFF that contains a collective pays a fixed multi-rank entry barrier
    (~50-80us measured) before the collective may start, and the global-scale
    AllGather gates everything downstream. Instead:

      launch A: each core DMAs its weight shard and reduces sum(|w|) to
                per-partition partials [128] -> returned as its output.
      host:     concatenates the 8 partial vectors into one [1024] array
                (pure layout - zero host arithmetic) and passes it back as a
                replicated *input* of launch B.
      launch B: the partials are available at t=0, so the total/threshold are
                ready within ~2us; quantization chases the (re-)DMA of the
                weight shard and the matmuls start ~4us into the launch.

    All arithmetic - including the final 1024-element combine (DVE reduce +
    ones-matmul broadcast) - happens on device.

Launch-B pipeline per core:
  1. DMA partials, 8-per-partition reduce, ones-matmul -> total broadcast to
     all 128 partitions; -scale = total * -2^-24 and thresh = total * 2^-25
     + eps/2 (bit-identical to 0.5*(mean+eps): power-of-2 scaling commutes
     with fp32 rounding).
  2. Ternary-quantize the shard to bf16 in 2 DVE ops per 128-row k-tile as it
     arrives from HBM, producing the NEGATED pattern (w<-thresh)-(w>thresh);
     the negation is undone by multiplying the output by -scale (both exact).
  3. 1024 accumulating matmuls: stationary = x.T tile [128k x 128t] (bf16),
     moving = quantized w.T k-slab [128k x 512o], accumulated over the 32
     k-tiles into PSUM banks (one per 128-token tile). Token tiles run in
     groups of 4 banks with the other 4 evacuating concurrently (ping-pong);
     evacuation is a DVE copy fused with the multiply by -scale.

Numerics: x is rounded to bf16 once (host side); everything else accumulates
in fp32 (PSUM) and the ternary weights are exact in bf16, so the end-to-end
error is ~1.7e-3 relative (bf16 input rounding), far inside the usual gates.
"""

import numpy as np
import ml_dtypes

import concourse.bacc as bacc
import concourse.mybir as mybir
import concourse.tile as tile
from concourse.bass_utils import run_bass_kernel_spmd

# Problem geometry (hardcoded per the contract).
B, S = 2, 2048
D_IN = 4096
D_OUT = 4096
N_CORES = 8

P = 128                      # SBUF/PSUM partitions
TOK = B * S                  # 4096 tokens
O_SHARD = D_OUT // N_CORES   # 512 output features per core
KT = D_IN // P               # 32 contraction k-tiles
TT = TOK // P                # 32 token tiles
NBANKS = 4                   # PSUM banks per token-tile group (4+4 ping-pong)
NG = TT // NBANKS            # 8 token-tile groups
GCOLS = P * NBANKS           # 512 tokens per group

F32 = mybir.dt.float32
BF16 = mybir.dt.bfloat16

EPS = np.float32(1e-8)
HALF_EPS = float(np.float32(0.5) * EPS)          # exact
NEG_INV_N = float(-np.float32(2.0 ** -24))       # -1/(4096*4096), exact
HALF_INV_N = float(np.float32(2.0 ** -25))


def _build_program_a():
    """Launch A: per-core per-partition sum(|w shard|) -> part [128, 1]."""
    nc = bacc.Bacc("TRN2", target_bir_lowering=False, debug=False,
                   num_devices=N_CORES)
    wt = nc.dram_tensor("wt", [D_IN, O_SHARD], F32, kind="ExternalInput")
    part = nc.dram_tensor("part", [P, 1], F32, kind="ExternalOutput")

    with tile.TileContext(nc) as tc:
        with (
            tc.tile_pool(name="wf", bufs=6) as wf,
            tc.tile_pool(name="small", bufs=1) as small,
        ):
            partials = small.tile([P, KT], F32)
            for k in range(KT):
                wtile = wf.tile([P, O_SHARD], F32, tag="w", name=f"w_{k}")
                nc.sync.dma_start(wtile[:], wt[k * P:(k + 1) * P, :])
                nc.vector.tensor_reduce(
                    partials[:, k:k + 1], wtile[:],
                    axis=mybir.AxisListType.X, op=mybir.AluOpType.add,
                    apply_absolute_value=True,
                )
            partial1 = small.tile([P, 1], F32)
            nc.vector.tensor_reduce(
                partial1[:, 0:1], partials[:, :],
                axis=mybir.AxisListType.X, op=mybir.AluOpType.add,
            )
            nc.sync.dma_start(part[:, :], partial1[:, 0:1])

    nc.compile()
    return nc


def _build_program_b():
    """Launch B: quantize + matmul, with all cores' partials as an input."""
    nc = bacc.Bacc("TRN2", target_bir_lowering=False, debug=False,
                   num_devices=N_CORES)

    xt = nc.dram_tensor("xt", [D_IN, TOK], BF16, kind="ExternalInput")
    wt = nc.dram_tensor("wt", [D_IN, O_SHARD], F32, kind="ExternalInput")
    parts = nc.dram_tensor("parts", [N_CORES * P, 1], F32, kind="ExternalInput")
    out = nc.dram_tensor("out", [TOK, O_SHARD], F32, kind="ExternalOutput")

    with tile.TileContext(nc) as tc:
        with (
            tc.tile_pool(name="const", bufs=1) as const,
            tc.tile_pool(name="wf", bufs=1) as wf,
            tc.tile_pool(name="wq", bufs=1) as wqp,
            tc.tile_pool(name="small", bufs=1) as small,
            tc.tile_pool(name="qtmp", bufs=4) as qtmp,
            tc.tile_pool(name="xp", bufs=8) as xp,
            tc.tile_pool(name="op", bufs=4) as op,
            tc.tile_pool(name="ps", bufs=8, space="PSUM") as ps,
        ):
            ones_sb = const.tile([P, P], F32)
            nc.vector.memset(ones_sb[:], 1.0)

            # ---- global scale from the precomputed partials ----------------
            # The 1024 values are summed order-agnostically: partition q takes
            # the 8 contiguous values [q*8, q*8+8), reduces them, and the
            # ones-matmul folds the 128 per-partition sums into the total,
            # broadcast to all partitions.
            gpart = small.tile([P, N_CORES], F32)
            nc.sync.dma_start(
                gpart[:, :], parts.rearrange("(p r) c -> p (r c)", r=N_CORES))
            gpart1 = small.tile([P, 1], F32)
            nc.vector.tensor_reduce(
                gpart1[:, 0:1], gpart[:, :],
                axis=mybir.AxisListType.X, op=mybir.AluOpType.add)
            psB = ps.tile([P, 512], F32, tag="acc", name="ps_bcast")
            nc.tensor.matmul(psB[:, 0:1], ones_sb[:, :], gpart1[:, 0:1],
                             start=True, stop=True)

            nscale_sb = small.tile([P, 1], F32)
            thresh_sb = small.tile([P, 1], F32)
            nthresh_sb = small.tile([P, 1], F32)
            nc.vector.tensor_scalar_mul(nscale_sb[:, 0:1], psB[:, 0:1], NEG_INV_N)
            nc.vector.tensor_scalar(
                thresh_sb[:, 0:1], psB[:, 0:1], HALF_INV_N, HALF_EPS,
                mybir.AluOpType.mult, mybir.AluOpType.add,
            )
            nc.vector.tensor_scalar_mul(nthresh_sb[:, 0:1], thresh_sb[:, 0:1], -1.0)

            # ---- DMA shard + ternary quantize -> bf16 NEGATED {-1, 0, +1} --
            # wq = (w < -thresh) - (w > thresh) = -ternary(w); undone by -scale.
            # The x tiles for the FIRST token group are DMA'd interleaved with
            # the w k-tiles so the matmuls can start as soon as k-tile 0 is
            # quantized (~4us in) instead of after the whole shard transfer.
            wt_sb = wf.tile([P, KT, O_SHARD], F32)
            wq_sb = wqp.tile([P, KT, O_SHARD], BF16)
            xg0 = []
            for k in range(KT):
                nc.sync.dma_start(wt_sb[:, k, :], wt[k * P:(k + 1) * P, :])
                xt_t = xp.tile([P, GCOLS], BF16, tag="xt", name=f"xt_0_{k}")
                nc.sync.dma_start(xt_t[:], xt[k * P:(k + 1) * P, 0:GCOLS])
                xg0.append(xt_t)
                pos = qtmp.tile([P, O_SHARD], BF16, tag="pos", name=f"pos_{k}")
                nc.vector.tensor_scalar(
                    pos[:], wt_sb[:, k, :], thresh_sb[:, 0:1], None,
                    mybir.AluOpType.is_gt,
                )
                nc.vector.scalar_tensor_tensor(
                    wq_sb[:, k, :], wt_sb[:, k, :], nthresh_sb[:, 0:1], pos[:],
                    mybir.AluOpType.is_lt, mybir.AluOpType.subtract,
                )

            # ---- main matmul: out[t, o] = sum_k xt[k, t] * wq[k, o] --------
            for g in range(NG):
                psums = [ps.tile([P, 512], F32, tag="acc", name=f"acc_{g}_{t}")
                         for t in range(NBANKS)]
                for k in range(KT):
                    if g == 0:
                        xt_t = xg0[k]
                    else:
                        xt_t = xp.tile([P, GCOLS], BF16, tag="xt",
                                       name=f"xt_{g}_{k}")
                        nc.sync.dma_start(
                            xt_t[:],
                            xt[k * P:(k + 1) * P, g * GCOLS:(g + 1) * GCOLS],
                        )
                    for t in range(NBANKS):
                        nc.tensor.matmul(
                            psums[t][:, :O_SHARD],
                            xt_t[:, t * P:(t + 1) * P],
                            wq_sb[:, k, :],
                            start=(k == 0), stop=(k == KT - 1),
                        )
                for t in range(NBANKS):
                    ot = op.tile([P, O_SHARD], F32, tag="ot", name=f"ot_{g}_{t}")
                    nc.vector.tensor_scalar_mul(
                        ot[:], psums[t][:, :O_SHARD], nscale_sb[:, 0:1])
                    row = (g * NBANKS + t) * P
                    nc.sync.dma_start(out[row:row + P, :], ot[:])

    nc.compile()
    return nc


_CACHE = {}


def _get_programs():
    if "a" not in _CACHE:
        _CACHE["a"] = _build_program_a()
        _CACHE["b"] = _build_program_b()
    return _CACHE["a"], _CACHE["b"]


def _shard_inputs(input: np.ndarray, weight: np.ndarray):
    input = np.asarray(input, dtype=np.float32)
    weight = np.asarray(weight, dtype=np.float32)
    x2d = np.ascontiguousarray(input.reshape(TOK, D_IN))
    xt_np = np.ascontiguousarray(x2d.T).astype(ml_dtypes.bfloat16)
    wT = np.ascontiguousarray(weight.T)          # [d_in, d_out] fp32
    w_shards = [np.ascontiguousarray(wT[:, c * O_SHARD:(c + 1) * O_SHARD])
                for c in range(N_CORES)]
    return xt_np, w_shards


def run_device(input: np.ndarray, weight: np.ndarray,
               spmd_a: dict | None = None, spmd_b: dict | None = None):
    """Run the two-launch sharded kernel.

    Returns (full_output, results_a, results_b)."""
    nc_a, nc_b = _get_programs()
    xt_np, w_shards = _shard_inputs(input, weight)
    cores = list(range(N_CORES))

    res_a = run_bass_kernel_spmd(
        nc_a, [{"wt": w_shards[c]} for c in cores], cores, **(spmd_a or {}))
    # Host-side gather/re-shard of the partials: concatenation only.
    parts = np.ascontiguousarray(
        np.concatenate([res_a.results[c]["part"] for c in cores], axis=0))

    res_b = run_bass_kernel_spmd(
        nc_b,
        [{"xt": xt_np, "wt": w_shards[c], "parts": parts} for c in cores],
        cores, **(spmd_b or {}))

    shards = [res_b.results[c]["out"] for c in cores]
    full = np.concatenate(shards, axis=1).reshape(B, S, D_OUT)
    return np.ascontiguousarray(full.astype(np.float32)), res_a, res_b


def kernel(input: np.ndarray, weight: np.ndarray) -> np.ndarray:
    out, _, _ = run_device(input, weight)
    return out


# revision 15
# speedup vs baseline: 1.2648x; 1.0423x over previous
"""BitLinear (BitNet b1.58 ternary-weight linear) Trainium2 kernel.

Reference computation:
    scale = mean(|w|)                      # global scalar over the FULL weight
    w_q   = round(clip(w / (scale+1e-8), -1, 1)) * scale    # ternary {-1,0,1}*scale
    out   = einsum('bsi,oi->bso', x, w_q)  # x @ w_q.T

Sharding (8 NeuronCores, tensor-parallel on out_features):
    core c receives:
      xt  [4096, 4096] bf16  = x.reshape(4096,4096).T   (replicated; [d_in, tok])
      wt  [4096,  512] f32   = w.T[:, c*512:(c+1)*512]  ([d_in, d_out/8] shard)
    and produces:
      out [4096,  512] f32   = (x @ w_q.T)[:, c*512:(c+1)*512]

Two collective-free launches instead of one collective kernel:
    A NEFF that contains a collective pays a fixed multi-rank entry barrier
    (~50-80us measured) before the collective may start, and the global-scale
    AllGather gates everything downstream. Instead:

      launch A: each core DMAs its weight shard and reduces sum(|w|) to
                per-partition partials [128] -> returned as its output.
      host:     concatenates the 8 partial vectors into one [1024] array
                (pure layout - zero host arithmetic) and passes it back as a
                replicated *input* of launch B.
      launch B: the partials are available at t=0, so the total/threshold are
                ready within ~2us; quantization chases the (re-)DMA of the
                weight shard and the matmuls start ~4us into the launch.

    All arithmetic - including the final 1024-element combine (DVE reduce +
    ones-matmul broadcast) - happens on device.

Launch-B pipeline per core:
  1. DMA partials, 8-per-partition reduce, ones-matmul -> total broadcast to
     all 128 partitions; -scale = total * -2^-24 and thresh = total * 2^-25
     + eps/2 (bit-identical to 0.5*(mean+eps): power-of-2 scaling commutes
     with fp32 rounding).
  2. Ternary-quantize the shard to bf16 in 2 DVE ops per 128-row k-tile as it
     arrives from HBM, producing the NEGATED pattern (w<-thresh)-(w>thresh);
     the negation is undone by multiplying the output by -scale (both exact).
  3. 1024 accumulating matmuls: stationary = x.T tile [128k x 128t] (bf16),
     moving = quantized w.T k-slab [128k x 512o], accumulated over the 32
     k-tiles into PSUM banks (one per 128-token tile). Token tiles run in
     groups of 4 banks with the other 4 evacuating concurrently (ping-pong);
     evacuation is a DVE copy fused with the multiply by -scale.

Numerics: x is rounded to bf16 once (host side); everything else accumulates
in fp32 (PSUM) and the ternary weights are exact in bf16, so the end-to-end
error is ~1.7e-3 relative (bf16 input rounding), far inside the usual gates.
"""

import numpy as np
import ml_dtypes

import concourse.bacc as bacc
import concourse.mybir as mybir
import concourse.tile as tile
from concourse.bass_utils import run_bass_kernel_spmd

# Problem geometry (hardcoded per the contract).
B, S = 2, 2048
D_IN = 4096
D_OUT = 4096
N_CORES = 8

P = 128                      # SBUF/PSUM partitions
TOK = B * S                  # 4096 tokens
O_SHARD = D_OUT // N_CORES   # 512 output features per core
KT = D_IN // P               # 32 contraction k-tiles
TT = TOK // P                # 32 token tiles
NBANKS = 4                   # PSUM banks per token-tile group (4+4 ping-pong)
NG = TT // NBANKS            # 8 token-tile groups
GCOLS = P * NBANKS           # 512 tokens per group

F32 = mybir.dt.float32
BF16 = mybir.dt.bfloat16

EPS = np.float32(1e-8)
HALF_EPS = float(np.float32(0.5) * EPS)          # exact
NEG_INV_N = float(-np.float32(2.0 ** -24))       # -1/(4096*4096), exact
HALF_INV_N = float(np.float32(2.0 ** -25))


def _build_program_a():
    """Launch A: per-core per-partition sum(|w shard|) -> part [128, 1]."""
    nc = bacc.Bacc("TRN2", target_bir_lowering=False, debug=False,
                   num_devices=N_CORES)
    wt = nc.dram_tensor("wt", [D_IN, O_SHARD], F32, kind="ExternalInput")
    part = nc.dram_tensor("part", [P, 1], F32, kind="ExternalOutput")

    with tile.TileContext(nc) as tc:
        with (
            tc.tile_pool(name="wf", bufs=6) as wf,
            tc.tile_pool(name="small", bufs=1) as small,
        ):
            partials = small.tile([P, KT], F32)
            for k in range(KT):
                wtile = wf.tile([P, O_SHARD], F32, tag="w", name=f"w_{k}")
                nc.sync.dma_start(wtile[:], wt[k * P:(k + 1) * P, :])
                nc.vector.tensor_reduce(
                    partials[:, k:k + 1], wtile[:],
                    axis=mybir.AxisListType.X, op=mybir.AluOpType.add,
                    apply_absolute_value=True,
                )
            partial1 = small.tile([P, 1], F32)
            nc.vector.tensor_reduce(
                partial1[:, 0:1], partials[:, :],
                axis=mybir.AxisListType.X, op=mybir.AluOpType.add,
            )
            nc.sync.dma_start(part[:, :], partial1[:, 0:1])

    nc.compile()
    return nc


def _build_program_b():
    """Launch B: quantize + matmul, with all cores' partials as an input."""
    nc = bacc.Bacc("TRN2", target_bir_lowering=False, debug=False,
                   num_devices=N_CORES)

    xt = nc.dram_tensor("xt", [D_IN, TOK], BF16, kind="ExternalInput")
    wt = nc.dram_tensor("wt", [D_IN, O_SHARD], F32, kind="ExternalInput")
    parts = nc.dram_tensor("parts", [N_CORES * P, 1], F32, kind="ExternalInput")
    out = nc.dram_tensor("out", [TOK, O_SHARD], F32, kind="ExternalOutput")

    with tile.TileContext(nc) as tc:
        with (
            tc.tile_pool(name="const", bufs=1) as const,
            tc.tile_pool(name="wf", bufs=1) as wf,
            tc.tile_pool(name="wq", bufs=1) as wqp,
            tc.tile_pool(name="small", bufs=1) as small,
            tc.tile_pool(name="qtmp", bufs=4) as qtmp,
            tc.tile_pool(name="xp", bufs=8) as xp,
            tc.tile_pool(name="op", bufs=4) as op,
            tc.tile_pool(name="ps", bufs=8, space="PSUM") as ps,
        ):
            ones_sb = const.tile([P, P], F32)
            nc.vector.memset(ones_sb[:], 1.0)

            # ---- global scale from the precomputed partials ----------------
            # The 1024 values are summed order-agnostically: partition q takes
            # the 8 contiguous values [q*8, q*8+8), reduces them, and the
            # ones-matmul folds the 128 per-partition sums into the total,
            # broadcast to all partitions.
            gpart = small.tile([P, N_CORES], F32)
            nc.sync.dma_start(
                gpart[:, :], parts.rearrange("(p r) c -> p (r c)", r=N_CORES))
            gpart1 = small.tile([P, 1], F32)
            nc.vector.tensor_reduce(
                gpart1[:, 0:1], gpart[:, :],
                axis=mybir.AxisListType.X, op=mybir.AluOpType.add)
            psB = ps.tile([P, 512], F32, tag="acc", name="ps_bcast")
            nc.tensor.matmul(psB[:, 0:1], ones_sb[:, :], gpart1[:, 0:1],
                             start=True, stop=True)

            nscale_sb = small.tile([P, 1], F32)
            thresh_sb = small.tile([P, 1], F32)
            nthresh_sb = small.tile([P, 1], F32)
            nc.vector.tensor_scalar_mul(nscale_sb[:, 0:1], psB[:, 0:1], NEG_INV_N)
            nc.vector.tensor_scalar(
                thresh_sb[:, 0:1], psB[:, 0:1], HALF_INV_N, HALF_EPS,
                mybir.AluOpType.mult, mybir.AluOpType.add,
            )
            nc.vector.tensor_scalar_mul(nthresh_sb[:, 0:1], thresh_sb[:, 0:1], -1.0)

            # ---- DMA shard + ternary quantize -> bf16 NEGATED {-1, 0, +1} --
            # wq = (w < -thresh) - (w > thresh) = -ternary(w); undone by -scale.
            # The x tiles for the FIRST token group are DMA'd interleaved with
            # the w k-tiles so the matmuls can start as soon as k-tile 0 is
            # quantized instead of after the whole shard transfer.
            #
            # Group sizing: during group 0 the DMA must feed BOTH the w shard
            # (256KB/k for quantization) and the x stream, so group 0 uses all
            # 8 PSUM banks (1024 tokens -> 8 matmuls = 1.7us of PE work per
            # k-step, matching ~300GB/s of DMA demand). Once the quantized
            # shard is resident, only x flows and the remaining 3072 tokens
            # run as 4-bank groups with the other banks evacuating (ping-pong).
            GROUPS = [(0, 8)] + [(1024 + i * 512, 4) for i in range(6)]
            wt_sb = wf.tile([P, KT, O_SHARD], F32)
            wq_sb = wqp.tile([P, KT, O_SHARD], BF16)
            xg0 = []
            for k in range(KT):
                nc.sync.dma_start(wt_sb[:, k, :], wt[k * P:(k + 1) * P, :])
                xt_t = xp.tile([P, 8 * P], BF16, tag="xt", name=f"xt_0_{k}")
                nc.sync.dma_start(xt_t[:], xt[k * P:(k + 1) * P, 0:8 * P])
                xg0.append(xt_t)
                pos = qtmp.tile([P, O_SHARD], BF16, tag="pos", name=f"pos_{k}")
                nc.vector.tensor_scalar(
                    pos[:], wt_sb[:, k, :], thresh_sb[:, 0:1], None,
                    mybir.AluOpType.is_gt,
                )
                nc.vector.scalar_tensor_tensor(
                    wq_sb[:, k, :], wt_sb[:, k, :], nthresh_sb[:, 0:1], pos[:],
                    mybir.AluOpType.is_lt, mybir.AluOpType.subtract,
                )

            # ---- main matmul: out[t, o] = sum_k xt[k, t] * wq[k, o] --------
            for g, (col0, nb) in enumerate(GROUPS):
                psums = [ps.tile([P, 512], F32, tag="acc", name=f"acc_{g}_{t}")
                         for t in range(nb)]
                for k in range(KT):
                    if g == 0:
                        xt_t = xg0[k]
                    else:
                        xt_t = xp.tile([P, nb * P], BF16, tag="xt",
                                       name=f"xt_{g}_{k}")
                        nc.sync.dma_start(
                            xt_t[:],
                            xt[k * P:(k + 1) * P, col0:col0 + nb * P],
                        )
                    for t in range(nb):
                        nc.tensor.matmul(
                            psums[t][:, :O_SHARD],
                            xt_t[:, t * P:(t + 1) * P],
                            wq_sb[:, k, :],
                            start=(k == 0), stop=(k == KT - 1),
                        )
                for t in range(nb):
                    ot = op.tile([P, O_SHARD], F32, tag="ot", name=f"ot_{g}_{t}")
                    nc.vector.tensor_scalar_mul(
                        ot[:], psums[t][:, :O_SHARD], nscale_sb[:, 0:1])
                    row = col0 + t * P
                    nc.sync.dma_start(out[row:row + P, :], ot[:])

    nc.compile()
    return nc


_CACHE = {}


def _get_programs():
    if "a" not in _CACHE:
        _CACHE["a"] = _build_program_a()
        _CACHE["b"] = _build_program_b()
    return _CACHE["a"], _CACHE["b"]


def _shard_inputs(input: np.ndarray, weight: np.ndarray):
    input = np.asarray(input, dtype=np.float32)
    weight = np.asarray(weight, dtype=np.float32)
    x2d = np.ascontiguousarray(input.reshape(TOK, D_IN))
    xt_np = np.ascontiguousarray(x2d.T).astype(ml_dtypes.bfloat16)
    wT = np.ascontiguousarray(weight.T)          # [d_in, d_out] fp32
    w_shards = [np.ascontiguousarray(wT[:, c * O_SHARD:(c + 1) * O_SHARD])
                for c in range(N_CORES)]
    return xt_np, w_shards


def run_device(input: np.ndarray, weight: np.ndarray,
               spmd_a: dict | None = None, spmd_b: dict | None = None):
    """Run the two-launch sharded kernel.

    Returns (full_output, results_a, results_b)."""
    nc_a, nc_b = _get_programs()
    xt_np, w_shards = _shard_inputs(input, weight)
    cores = list(range(N_CORES))

    res_a = run_bass_kernel_spmd(
        nc_a, [{"wt": w_shards[c]} for c in cores], cores, **(spmd_a or {}))
    # Host-side gather/re-shard of the partials: concatenation only.
    parts = np.ascontiguousarray(
        np.concatenate([res_a.results[c]["part"] for c in cores], axis=0))

    res_b = run_bass_kernel_spmd(
        nc_b,
        [{"xt": xt_np, "wt": w_shards[c], "parts": parts} for c in cores],
        cores, **(spmd_b or {}))

    shards = [res_b.results[c]["out"] for c in cores]
    full = np.concatenate(shards, axis=1).reshape(B, S, D_OUT)
    return np.ascontiguousarray(full.astype(np.float32)), res_a, res_b


def kernel(input: np.ndarray, weight: np.ndarray) -> np.ndarray:
    out, _, _ = run_device(input, weight)
    return out
